# revision 13
# baseline (speedup 1.0000x reference)
"""BoxConv2d Trainium2 kernel (8 NeuronCores, SPMD).

Math: the reference computes, per output channel k = (c, f),
    out[b,k] = interp-row(I) diff, then interp-col diff
where I is the zero-padded integral image of input[b,c].  That whole
pipeline (integral image + fractional box-edge interpolation) is linear
in the input and separable, so it collapses to two dense 128x128
matrix products per image:

    out[b,k] = A_k @ x[b,c] @ B_k^T

with banded "pixel overlap" matrices
    A_k[xo, a] = clamp(xo - a + x_max_k + 1, 0, 1)
                 - clamp(xo - a + x_min_k, 0, 1)
(the overlap length between the box row extent [xo+x_min, xo+x_max+1]
and the pixel row [a, a+1]), and likewise B_k for columns.  A/B are
built on the host from the tiny (C,F) box params; the device does pure
128-contraction matmuls on the PE array.

Sharding: the K = C*F = 128 output channels are split across 8 cores
(16 channels = 4 in_planes per core), so each core reads only its own
4 input planes and input reads are not duplicated chip-wide.

v2 changes vs the 52us baseline (trace-driven):
  * everything bf16 on the wire (x, at, bt, V, out) -- halves the DMA
    byte volume, which the trace showed saturating the per-core DMA
    bus (~320 GB/s) for ~40us.  PSUM accumulation stays fp32; measured
    l2 rel err ~2e-3 vs the 2e-2 gate.
  * pass 2 streams all 8 batches per (c,f) in one N=1024 matmul, so
    each B_k weight matrix is loaded once (16 LDWEIGHTS instead of 32).
  * PSUM tiles are [128,1024] (2 banks) so PSUM->SBUF copies move 1024
    columns per instruction; the 32 copies are round-robined across
    Scalar/Vector/GpSimd so no single engine serializes.
  * all DMA stays on the Sync HWDGE queue (which by itself sustains
    ~360 GB/s), ordered so the first pass-1 matmul has its operands
    after ~0.9us instead of 12.5us.

Numerics: bf16 inputs with fp32 accumulation; l2 relative error vs the
fp32 reference is ~2e-3.  Set BOXCONV_MM_DT=f32r/f32 for the previous
higher-precision (but slower) paths.
"""

import os
import sys

if "/opt/trn_rl_repo" not in sys.path:
    sys.path.insert(0, "/opt/trn_rl_repo")

import ml_dtypes
import numpy as np

import concourse.bass as bass  # noqa: F401
import concourse.mybir as mybir
import concourse.tile as tile
from concourse import bacc
from concourse.bass_utils import run_bass_kernel_spmd

B, C, F, H, W = 8, 32, 4, 128, 128
NCORES = 8
CPC = C // NCORES  # in_planes per core
KPC = CPC * F      # output channels per core

_DT = mybir.dt.bfloat16
_NP_DT = ml_dtypes.bfloat16

_NC_CACHE = {}
LAST_RESULT = None


def _build_nc():
    nc = bacc.Bacc(
        "TRN2", target_bir_lowering=False, debug=False, num_devices=NCORES
    )
    # x[a, (b, c, j)]: per-(b,c) lhsT slice is [128, 128]; per-b DMA rows
    # are 512 elems = 1KB contiguous in DRAM (>=512B descriptor floor)
    x_p = nc.declare_dram_parameter("x", [H, B * CPC * W], _DT, isOutput=False)
    # at[a, (c, f, xo)] / bt[j, (c, f, yo)]: per-c DMA rows are 1KB
    at_p = nc.declare_dram_parameter(
        "at", [H, CPC * F * H], _DT, isOutput=False)
    bt_p = nc.declare_dram_parameter(
        "bt", [W, CPC * F * W], _DT, isOutput=False)
    # outT[kl, yo, (b, xo)]: one contiguous 256KB block per (c,f)
    out_p = nc.declare_dram_parameter(
        "outT", [KPC, W, B * H], _DT, isOutput=True)

    with tile.TileContext(nc) as tc:
        with (
            tc.tile_pool(name="const", bufs=1) as cpool,
            tc.tile_pool(name="vall", bufs=4) as vpool,
            tc.tile_pool(name="osb", bufs=6) as opool,
            tc.tile_pool(name="pv", bufs=2, space="PSUM") as pvpool,
            tc.tile_pool(name="po", bufs=2, space="PSUM") as popool,
        ):
            # Dependency tracking is tile-granular and queued DMAs finish
            # near-together, so the critical-path operands (at[c0], x[b0-3])
            # get small dedicated DMAs at the head of the Sync queue; the
            # remaining loads go on the Scalar HWDGE queue (idle early) so
            # output stores never queue behind them on Sync.
            at_t = [cpool.tile([128, F * H], _DT, name=f"at{c}",
                               tag=f"at{c}") for c in range(CPC)]
            bt_t = [cpool.tile([128, F * W], _DT, name=f"bt{c}",
                               tag=f"bt{c}") for c in range(CPC)]
            x_t = [cpool.tile([128, 2 * CPC * W], _DT, name=f"x{i}",
                              tag=f"x{i}") for i in range(B // 2)]

            def at_c(c):
                return at_t[c][:]

            def bt_c(c, f):
                return bt_t[c][:, f * W:(f + 1) * W]

            def x_bc(b, c):
                o = ((b % 2) * CPC + c) * W
                return x_t[b // 2][:, o:o + W]

            # The DMA engines fair-share across all in-flight transfers,
            # so a load completes with everything issued alongside it.
            # Issue strictly in need order on one queue (whose ~0.65us
            # per-instruction handoff self-throttles the flight depth);
            # only the last-needed bt tiles ride the Scalar queue.
            at_r = at_p[:].rearrange("a (c fx) -> a c fx", c=CPC)
            bt_r = bt_p[:].rearrange("j (c fy) -> j c fy", c=CPC)
            x_r = x_p[:].rearrange("a (p bcj) -> a p bcj", p=B // 2)
            nc.sync.dma_start(at_t[0][:], at_r[:, 0])
            for i in range(B // 2):
                nc.sync.dma_start(x_t[i][:], x_r[:, i])
            nc.sync.dma_start(at_t[1][:], at_r[:, 1])
            nc.sync.dma_start(bt_t[0][:], bt_r[:, 0])
            nc.sync.dma_start(at_t[2][:], at_r[:, 2])
            nc.sync.dma_start(bt_t[1][:], bt_r[:, 1])
            nc.sync.dma_start(at_t[3][:], at_r[:, 3])
            nc.scalar.dma_start(bt_t[2][:], bt_r[:, 2])
            nc.scalar.dma_start(bt_t[3][:], bt_r[:, 3])

            # only Scalar and Vector can read PSUM on TRN2; alternate the
            # PSUM->SBUF copies, slightly favoring the faster Activation
            # engine (17:15 over the 32 copies)
            cp_i = [0]

            def copy(dst, src):
                i = cp_i[0]
                cp_i[0] += 1
                if i % 2 == 1 and i < 30:
                    nc.vector.tensor_copy(dst, src)
                else:
                    nc.scalar.copy(dst, src)

            v_all = [None] * CPC

            def emit_pass1(c, bp):
                # two 512-col matmuls into one 2-bank PSUM tile, then one
                # 1024-col copy into V[c][j, (f, b, xo)]
                if bp == 0:
                    v_all[c] = vpool.tile(
                        [128, F * B * H], _DT, name=f"v{c}", tag="vall")
                v_ps = pvpool.tile([128, 2 * F * H], mybir.dt.float32,
                                   name=f"vps{c}{bp}", tag="vps")
                for i in range(2):
                    b = 2 * bp + i
                    nc.tensor.matmul(
                        v_ps[:, i * F * H:(i + 1) * F * H],
                        lhsT=x_bc(b, c),
                        rhs=at_c(c),
                        start=True,
                        stop=True,
                    )
                src = v_ps[:].rearrange("p (i f xo) -> p i f xo", i=2, f=F)
                dst = v_all[c][:].rearrange(
                    "p (f b xo) -> p f b xo", f=F, b=B)
                copy(dst[:, :, 2 * bp:2 * bp + 2, :],
                     src.rearrange("p i f xo -> p f i xo"))

            def emit_pass2(c, f):
                kl = c * F + f
                # O[yo, (b, xo)] for all 8 batches: 2x N=512 matmuls
                # (ISA caps the moving dim at 512) into one 2-bank tile
                o_ps = popool.tile([128, B * H], mybir.dt.float32,
                                   name=f"ops{kl}", tag="ops")
                for i in range(2):
                    nc.tensor.matmul(
                        o_ps[:, i * 512:(i + 1) * 512],
                        lhsT=bt_c(c, f),
                        rhs=v_all[c][:, f * B * H + i * 512:
                                     f * B * H + (i + 1) * 512],
                        start=True,
                        stop=True,
                    )
                o_sb = opool.tile([128, B * H], _DT,
                                  name=f"osb{kl}", tag="osb")
                copy(o_sb[:], o_ps[:])
                nc.sync.dma_start(out_p[kl], o_sb[:])

            # software pipeline: pass 2 of channel c-1 interleaves with
            # pass 1 of channel c at matching granularity, keeping the PE
            # dense and the output DMA stream flowing from ~6us onward
            for bp in range(B // 2):
                emit_pass1(0, bp)
            for c in range(1, CPC):
                for k in range(4):
                    emit_pass2(c - 1, k)
                    emit_pass1(c, k)
            for f in range(F):
                emit_pass2(CPC - 1, f)
    nc.finalize()
    return nc


def _get_nc():
    if "nc" not in _NC_CACHE:
        _NC_CACHE["nc"] = _build_nc()
    return _NC_CACHE["nc"]


def _overlap_mats(lo, hi):
    """(K, out, in) pixel-overlap matrices for a 128-wide axis."""
    t = np.arange(128, dtype=np.float64)
    d = t[:, None] - t[None, :]  # out - in
    lo = lo.astype(np.float64)[:, None, None]
    hi = hi.astype(np.float64)[:, None, None]
    m = np.clip(d[None] + hi + 1.0, 0.0, 1.0) - np.clip(d[None] + lo, 0.0, 1.0)
    return m.astype(np.float32)


def _make_in_maps(input, x_min, x_max, y_min, y_max):
    A = _overlap_mats(x_min.reshape(-1), x_max.reshape(-1))   # (K, xo, a)
    Bm = _overlap_mats(y_min.reshape(-1), y_max.reshape(-1))  # (K, yo, j)
    in_maps = []
    for m in range(NCORES):
        cs = slice(CPC * m, CPC * (m + 1))
        ks = slice(KPC * m, KPC * (m + 1))
        # x[a, (b, c, j)]
        xm = input[:, cs].transpose(2, 0, 1, 3).reshape(H, B * CPC * W)
        # at[a, (c, f, xo)] = A[k=c*F+f, xo, a]
        at = A[ks].reshape(CPC, F, H, H).transpose(3, 0, 1, 2)
        bt = Bm[ks].reshape(CPC, F, W, W).transpose(3, 0, 1, 2)
        in_maps.append({
            "x": np.ascontiguousarray(xm).astype(_NP_DT),
            "at": np.ascontiguousarray(
                at.reshape(H, CPC * F * H)).astype(_NP_DT),
            "bt": np.ascontiguousarray(
                bt.reshape(W, CPC * F * W)).astype(_NP_DT),
        })
    return in_maps


def _assemble(results):
    out = np.empty((B, C * F, H, W), np.float32)
    for m in range(NCORES):
        # outT[kl, yo, b, xo] -> out[b, kl, xo, yo]
        o = results[m]["outT"].reshape(KPC, W, B, H).astype(np.float32)
        out[:, KPC * m:KPC * (m + 1)] = o.transpose(2, 0, 3, 1)
    return out


def _run(inputs, trace=False):
    global LAST_RESULT
    nc = _get_nc()
    in_maps = _make_in_maps(**inputs)
    LAST_RESULT = run_bass_kernel_spmd(
        nc, in_maps, list(range(NCORES)), trace=trace
    )
    return _assemble(LAST_RESULT.results)


def kernel(input, x_min, x_max, y_min, y_max):
    return _run({
        "input": np.asarray(input, dtype=np.float32),
        "x_min": np.asarray(x_min, dtype=np.float32),
        "x_max": np.asarray(x_max, dtype=np.float32),
        "y_min": np.asarray(y_min, dtype=np.float32),
        "y_max": np.asarray(y_max, dtype=np.float32),
    })


# revision 14
# speedup vs baseline: 1.0543x; 1.0543x over previous
"""BoxConv2d Trainium2 kernel (8 NeuronCores, SPMD).

Math: the reference computes, per output channel k = (c, f),
    out[b,k] = interp-row(I) diff, then interp-col diff
where I is the zero-padded integral image of input[b,c].  That whole
pipeline (integral image + fractional box-edge interpolation) is linear
in the input and separable, so it collapses to two dense 128x128
matrix products per image:

    out[b,k] = A_k @ x[b,c] @ B_k^T

with banded "pixel overlap" matrices
    A_k[xo, a] = clamp(xo - a + x_max_k + 1, 0, 1)
                 - clamp(xo - a + x_min_k, 0, 1)
(the overlap length between the box row extent [xo+x_min, xo+x_max+1]
and the pixel row [a, a+1]), and likewise B_k for columns.  A/B are
built on the host from the tiny (C,F) box params; the device does pure
128-contraction matmuls on the PE array.

Sharding: the K = C*F = 128 output channels are split across 8 cores
(16 channels = 4 in_planes per core), so each core reads only its own
4 input planes and input reads are not duplicated chip-wide.

v2 changes vs the 52us baseline (trace-driven):
  * everything bf16 on the wire (x, at, bt, V, out) -- halves the DMA
    byte volume, which the trace showed saturating the per-core DMA
    bus (~320 GB/s) for ~40us.  PSUM accumulation stays fp32; measured
    l2 rel err ~2e-3 vs the 2e-2 gate.
  * pass 2 streams all 8 batches per (c,f) in one N=1024 matmul, so
    each B_k weight matrix is loaded once (16 LDWEIGHTS instead of 32).
  * PSUM tiles are [128,1024] (2 banks) so PSUM->SBUF copies move 1024
    columns per instruction; the 32 copies are round-robined across
    Scalar/Vector/GpSimd so no single engine serializes.
  * all DMA stays on the Sync HWDGE queue (which by itself sustains
    ~360 GB/s), ordered so the first pass-1 matmul has its operands
    after ~0.9us instead of 12.5us.

Numerics: bf16 inputs with fp32 accumulation; l2 relative error vs the
fp32 reference is ~2e-3.  Set BOXCONV_MM_DT=f32r/f32 for the previous
higher-precision (but slower) paths.
"""

import os
import sys

if "/opt/trn_rl_repo" not in sys.path:
    sys.path.insert(0, "/opt/trn_rl_repo")

import ml_dtypes
import numpy as np

import concourse.bass as bass  # noqa: F401
import concourse.mybir as mybir
import concourse.tile as tile
from concourse import bacc
from concourse.bass_utils import run_bass_kernel_spmd

B, C, F, H, W = 8, 32, 4, 128, 128
NCORES = 8
CPC = C // NCORES  # in_planes per core
KPC = CPC * F      # output channels per core

_DT = mybir.dt.bfloat16
_NP_DT = ml_dtypes.bfloat16

_NC_CACHE = {}
LAST_RESULT = None


def _build_nc():
    nc = bacc.Bacc(
        "TRN2", target_bir_lowering=False, debug=False, num_devices=NCORES
    )
    # x[a, (b, c, j)]: per-(b,c) lhsT slice is [128, 128]; per-b DMA rows
    # are 512 elems = 1KB contiguous in DRAM (>=512B descriptor floor)
    x_p = nc.declare_dram_parameter("x", [H, B * CPC * W], _DT, isOutput=False)
    # at[a, (c, f, xo)] / bt[j, (c, f, yo)]: per-c DMA rows are 1KB
    at_p = nc.declare_dram_parameter(
        "at", [H, CPC * F * H], _DT, isOutput=False)
    bt_p = nc.declare_dram_parameter(
        "bt", [W, CPC * F * W], _DT, isOutput=False)
    # outT[kl, yo, (b, xo)]: one contiguous 256KB block per (c,f)
    out_p = nc.declare_dram_parameter(
        "outT", [KPC, W, B * H], _DT, isOutput=True)

    with tile.TileContext(nc) as tc:
        with (
            tc.tile_pool(name="const", bufs=1) as cpool,
            tc.tile_pool(name="vall", bufs=4) as vpool,
            tc.tile_pool(name="osb", bufs=6) as opool,
            tc.tile_pool(name="pv", bufs=2, space="PSUM") as pvpool,
            tc.tile_pool(name="po", bufs=2, space="PSUM") as popool,
        ):
            # Dependency tracking is tile-granular and queued DMAs finish
            # near-together, so the critical-path operands (at[c0], x[b0-3])
            # get small dedicated DMAs at the head of the Sync queue; the
            # remaining loads go on the Scalar HWDGE queue (idle early) so
            # output stores never queue behind them on Sync.
            at_t = [cpool.tile([128, F * H], _DT, name=f"at{c}",
                               tag=f"at{c}") for c in range(CPC)]
            bt_t = [cpool.tile([128, F * W], _DT, name=f"bt{c}",
                               tag=f"bt{c}") for c in range(CPC)]
            x_t = [cpool.tile([128, 2 * CPC * W], _DT, name=f"x{i}",
                              tag=f"x{i}") for i in range(B // 2)]

            def at_c(c):
                return at_t[c][:]

            def bt_c(c, f):
                return bt_t[c][:, f * W:(f + 1) * W]

            def x_bc(b, c):
                o = ((b % 2) * CPC + c) * W
                return x_t[b // 2][:, o:o + W]

            # The DMA engines fair-share across all in-flight transfers,
            # so a load completes with everything issued alongside it.
            # Issue strictly in need order on one queue (whose ~0.65us
            # per-instruction handoff self-throttles the flight depth);
            # only the last-needed bt tiles ride the Scalar queue.
            at_r = at_p[:].rearrange("a (c fx) -> a c fx", c=CPC)
            bt_r = bt_p[:].rearrange("j (c fy) -> j c fy", c=CPC)
            x_r = x_p[:].rearrange("a (p bcj) -> a p bcj", p=B // 2)
            nc.sync.dma_start(at_t[0][:], at_r[:, 0])
            for i in range(B // 2):
                nc.sync.dma_start(x_t[i][:], x_r[:, i])
            nc.sync.dma_start(at_t[1][:], at_r[:, 1])
            nc.sync.dma_start(bt_t[0][:], bt_r[:, 0])
            nc.sync.dma_start(at_t[2][:], at_r[:, 2])
            nc.sync.dma_start(bt_t[1][:], bt_r[:, 1])
            nc.sync.dma_start(at_t[3][:], at_r[:, 3])
            nc.sync.dma_start(bt_t[2][:], bt_r[:, 2])
            nc.sync.dma_start(bt_t[3][:], bt_r[:, 3])

            # The core's DVFS grants full clock for one fixed ~20.5us
            # window starting ~9us after sustained PE activity.  Spin the
            # PE on dependency-free dummy matmuls from the moment the
            # preamble ends so the grant (and the 2x clock) arrives while
            # the real pipeline is still loading.
            warm = cpool.tile([128, 512], _DT, name="warm", tag="warm")
            nc.gpsimd.memset(warm[:], 0.0)
            for w in range(6):
                w_ps = pvpool.tile([128, 2 * F * H], mybir.dt.float32,
                                   name=f"wps{w}", tag="vps")
                nc.tensor.matmul(
                    w_ps[:, :512],
                    lhsT=warm[:, :128],
                    rhs=warm[:],
                    start=True,
                    stop=True,
                )

            # only Scalar and Vector can read PSUM on TRN2; alternate the
            # PSUM->SBUF copies, slightly favoring the faster Activation
            # engine (17:15 over the 32 copies)
            cp_i = [0]

            def copy(dst, src):
                i = cp_i[0]
                cp_i[0] += 1
                if i % 2 == 1 and i < 30:
                    nc.vector.tensor_copy(dst, src)
                else:
                    nc.scalar.copy(dst, src)

            v_all = [None] * CPC

            def emit_pass1(c, bp):
                # two 512-col matmuls into one 2-bank PSUM tile, then one
                # 1024-col copy into V[c][j, (f, b, xo)]
                if bp == 0:
                    v_all[c] = vpool.tile(
                        [128, F * B * H], _DT, name=f"v{c}", tag="vall")
                v_ps = pvpool.tile([128, 2 * F * H], mybir.dt.float32,
                                   name=f"vps{c}{bp}", tag="vps")
                for i in range(2):
                    b = 2 * bp + i
                    nc.tensor.matmul(
                        v_ps[:, i * F * H:(i + 1) * F * H],
                        lhsT=x_bc(b, c),
                        rhs=at_c(c),
                        start=True,
                        stop=True,
                    )
                src = v_ps[:].rearrange("p (i f xo) -> p i f xo", i=2, f=F)
                dst = v_all[c][:].rearrange(
                    "p (f b xo) -> p f b xo", f=F, b=B)
                copy(dst[:, :, 2 * bp:2 * bp + 2, :],
                     src.rearrange("p i f xo -> p f i xo"))

            def emit_pass2(c, f):
                kl = c * F + f
                # O[yo, (b, xo)] for all 8 batches: 2x N=512 matmuls
                # (ISA caps the moving dim at 512) into one 2-bank tile
                o_ps = popool.tile([128, B * H], mybir.dt.float32,
                                   name=f"ops{kl}", tag="ops")
                for i in range(2):
                    nc.tensor.matmul(
                        o_ps[:, i * 512:(i + 1) * 512],
                        lhsT=bt_c(c, f),
                        rhs=v_all[c][:, f * B * H + i * 512:
                                     f * B * H + (i + 1) * 512],
                        start=True,
                        stop=True,
                    )
                o_sb = opool.tile([128, B * H], _DT,
                                  name=f"osb{kl}", tag="osb")
                copy(o_sb[:], o_ps[:])
                nc.sync.dma_start(out_p[kl], o_sb[:])

            # software pipeline: pass 2 of channel c-1 interleaves with
            # pass 1 of channel c at matching granularity, keeping the PE
            # dense and the output DMA stream flowing from ~6us onward
            for bp in range(B // 2):
                emit_pass1(0, bp)
            for c in range(1, CPC):
                for k in range(4):
                    emit_pass2(c - 1, k)
                    emit_pass1(c, k)
            for f in range(F):
                emit_pass2(CPC - 1, f)
    nc.finalize()
    return nc


def _get_nc():
    if "nc" not in _NC_CACHE:
        _NC_CACHE["nc"] = _build_nc()
    return _NC_CACHE["nc"]


def _overlap_mats(lo, hi):
    """(K, out, in) pixel-overlap matrices for a 128-wide axis."""
    t = np.arange(128, dtype=np.float64)
    d = t[:, None] - t[None, :]  # out - in
    lo = lo.astype(np.float64)[:, None, None]
    hi = hi.astype(np.float64)[:, None, None]
    m = np.clip(d[None] + hi + 1.0, 0.0, 1.0) - np.clip(d[None] + lo, 0.0, 1.0)
    return m.astype(np.float32)


def _make_in_maps(input, x_min, x_max, y_min, y_max):
    A = _overlap_mats(x_min.reshape(-1), x_max.reshape(-1))   # (K, xo, a)
    Bm = _overlap_mats(y_min.reshape(-1), y_max.reshape(-1))  # (K, yo, j)
    in_maps = []
    for m in range(NCORES):
        cs = slice(CPC * m, CPC * (m + 1))
        ks = slice(KPC * m, KPC * (m + 1))
        # x[a, (b, c, j)]
        xm = input[:, cs].transpose(2, 0, 1, 3).reshape(H, B * CPC * W)
        # at[a, (c, f, xo)] = A[k=c*F+f, xo, a]
        at = A[ks].reshape(CPC, F, H, H).transpose(3, 0, 1, 2)
        bt = Bm[ks].reshape(CPC, F, W, W).transpose(3, 0, 1, 2)
        in_maps.append({
            "x": np.ascontiguousarray(xm).astype(_NP_DT),
            "at": np.ascontiguousarray(
                at.reshape(H, CPC * F * H)).astype(_NP_DT),
            "bt": np.ascontiguousarray(
                bt.reshape(W, CPC * F * W)).astype(_NP_DT),
        })
    return in_maps


def _assemble(results):
    out = np.empty((B, C * F, H, W), np.float32)
    for m in range(NCORES):
        # outT[kl, yo, b, xo] -> out[b, kl, xo, yo]
        o = results[m]["outT"].reshape(KPC, W, B, H).astype(np.float32)
        out[:, KPC * m:KPC * (m + 1)] = o.transpose(2, 0, 3, 1)
    return out


def _run(inputs, trace=False):
    global LAST_RESULT
    nc = _get_nc()
    in_maps = _make_in_maps(**inputs)
    LAST_RESULT = run_bass_kernel_spmd(
        nc, in_maps, list(range(NCORES)), trace=trace
    )
    return _assemble(LAST_RESULT.results)


def kernel(input, x_min, x_max, y_min, y_max):
    return _run({
        "input": np.asarray(input, dtype=np.float32),
        "x_min": np.asarray(x_min, dtype=np.float32),
        "x_max": np.asarray(x_max, dtype=np.float32),
        "y_min": np.asarray(y_min, dtype=np.float32),
        "y_max": np.asarray(y_max, dtype=np.float32),
    })
